# revision 9
# baseline (speedup 1.0000x reference)
"""AlphaRenderer v9 (C3): P128 input, int8 out, dual-ring per-slot feed.

Math: out = W @ e3m4(A/255-0.5) + 0.5*sum(W); weights pre-scaled by
OS=400 on host; og int8 = OS*out - OOFF (ACT/DVE round+saturate);
host un-scales. Table padded to 128 partitions so HWDGE DMAs spread
across all 16 SDMA engines (measured 399 vs 142 GB/s).

Schedule:
- sync ring: weight-prefix DMA first (bias matmuls start ~4us
  earlier), then even slots; scalar ring: odd slots. Per-ring FIFO
  completion -> two counting sems.
- Warmup NWARM matmuls bridge the pre-input idle window so the HAM
  clock gate (4096cy window) reaches 8/8 before real matmuls.
- Outputs: gpsimd ring even pairs, scalar odd, sync final; final
  pair in 4 column segments to cut the post-matmul tail. NOG=npairs
  og buffers -> no reuse gating; single out_done semaphore.
"""
from contextlib import ExitStack

import ml_dtypes
import numpy as np

import concourse.bass as bass
import concourse.mybir as mybir
from concourse.bass_utils import run_bass_kernel_spmd

BF16 = np.dtype(ml_dtypes.bfloat16)
E3M4 = np.dtype(ml_dtypes.float8_e3m4)

NCORES = 8
F = 100
C = 100
N = 4096
HW = 4096
TOPK = 20
KCAP = 64
NT = 512
PS = 1024
WTB = 2 * KCAP
P128 = 128

OS = 400.0       # output int8 scale
OOFF = 123.2     # og = OS*out - OOFF

_NC_CACHE: dict = {}
LAST_RESULT = None
NWARM = 25
WNT = 256


def _build(S):
    key = ("v11", S)
    if key in _NC_CACHE:
        return _NC_CACHE[key]
    dt8 = mybir.dt.float8e3
    dtb = mybir.dt.bfloat16
    npairs = (S + 1) // 2
    NOG = npairs
    ntiles = 4 * npairs
    nslots_of = lambda p: min(2, S - 2 * p)
    R = 64 * S
    W0 = S * WTB

    nc = bass.Bass("TRN2", target_bir_lowering=False, debug=False,
                   num_devices=NCORES)
    table = nc.dram_tensor("table", [P128, W0 + S * HW], dt8,
                           kind="ExternalInput").ap()
    out = nc.dram_tensor("out", [R, HW], mybir.dt.int8,
                         kind="ExternalOutput").ap()

    ctx = ExitStack()
    ones = ctx.enter_context(nc.sbuf_tensor("ones", [F, 1], dtb))
    gw = ctx.enter_context(nc.sbuf_tensor("gw", [F, NT], dtb))
    bs = ctx.enter_context(nc.sbuf_tensor("bs", [128, npairs],
                                          mybir.dt.float32))
    rt = ctx.enter_context(nc.sbuf_tensor("rt", [P128, W0 + S * HW], dt8))
    ogs = [ctx.enter_context(nc.sbuf_tensor(f"og{i}", [128, HW],
                                            mybir.dt.int8))
           for i in range(NOG)]
    pts = [ctx.enter_context(nc.psum_tensor(f"pt{i}", [128, PS],
                                            mybir.dt.float32))
           for i in range(4)]
    gw_sem = ctx.enter_context(nc.semaphore("gw_sem"))
    bmm_sem = ctx.enter_context(nc.semaphore("bmm_sem"))
    bias_sem = ctx.enter_context(nc.semaphore("bias_sem"))
    pre_sem = ctx.enter_context(nc.semaphore("pre_sem"))
    ine_sem = ctx.enter_context(nc.semaphore("ine_sem"))
    ino_sem = ctx.enter_context(nc.semaphore("ino_sem"))
    mm_sem = ctx.enter_context(nc.semaphore("mm_sem"))
    cpv = ctx.enter_context(nc.semaphore("cpv"))
    cps = ctx.enter_context(nc.semaphore("cps"))
    out_done = ctx.enter_context(nc.semaphore("out_done"))

    def wtap(s):
        return rt.ap()[:F, s * WTB:(s + 1) * WTB].bitcast(dtb)

    fin_half = nslots_of(npairs - 1) == 1

    def out_segs(p):
        if nslots_of(p) == 2:
            return [(0, HW, 4)]
        return [(j * PS, PS, j + 1) for j in range(4)]

    n_out = sum(len(out_segs(p)) for p in range(npairs))

    def issue_out(eng, p):
        ns = nslots_of(p)
        for (off, wid, hi4) in out_segs(p):
            if p == npairs - 1 and fin_half:
                eng.wait_ge(cpv, 2 * (npairs - 1) + hi4)
                eng.wait_ge(cps, 2 * (npairs - 1) + hi4)
            else:
                hi = 4 * p + hi4 - 1         # last tile index needed
                eng.wait_ge(cpv, hi // 2 + 1)
                eng.wait_ge(cps, (hi + 1) // 2)
            eng.dma_start(
                out[128 * p:128 * p + 64 * ns, off:off + wid],
                ogs[p % NOG].ap()[:64 * ns, off:off + wid]
            ).then_inc(out_done, 16)

    # pair -> out ring: 0=gpsimd 1=scalar 2=sync(final)
    def out_q(p):
        if p == npairs - 1:
            return 2
        return 0 if p % 2 == 0 else 1

    with nc.Block() as block:

        @block.sync
        def _(sync):
            sync.dma_start(rt[:, 0:W0 + HW],
                           table[:, 0:W0 + HW]).then_inc(pre_sem, 16)
            for s in range(2, S, 2):
                lo = W0 + s * HW
                sync.dma_start(rt[:, lo:lo + HW],
                               table[:, lo:lo + HW]).then_inc(ine_sem, 16)
            for p in range(npairs):
                if out_q(p) == 2:
                    issue_out(sync, p)

        @block.scalar
        def _(scalar):
            for s in range(1, S, 2):
                lo = W0 + s * HW
                scalar.dma_start(rt[:, lo:lo + HW],
                                 table[:, lo:lo + HW]).then_inc(ino_sem, 16)
            nmain = ntiles - 4 if fin_half else ntiles
            for t in range(1, nmain, 2):
                p, c = divmod(t, 4)
                ns = nslots_of(p)
                scalar.wait_ge(mm_sem, t + 1)
                if t == 1:
                    scalar.wait_ge(bias_sem, 1)
                og = ogs[p % NOG]
                scalar.activation(og.ap()[:64 * ns, c * PS:c * PS + PS],
                                  pts[c].ap()[:64 * ns, :],
                                  mybir.ActivationFunctionType.Identity,
                                  bias=bs.ap()[:64 * ns, p:p + 1],
                                  scale=1.0).then_inc(cps, 1)
                if c == 3 and out_q(p) == 1:
                    issue_out(scalar, p)
            if fin_half:
                pf = npairs - 1
                og = ogs[pf % NOG]
                for c in range(4):
                    scalar.wait_ge(mm_sem, 4 * pf + c + 1)
                    scalar.activation(
                        og.ap()[:64, c * PS + NT:c * PS + PS],
                        pts[c].ap()[:64, NT:PS],
                        mybir.ActivationFunctionType.Identity,
                        bias=bs.ap()[:64, pf:pf + 1],
                        scale=1.0).then_inc(cps, 1)

        @block.tensor
        def _(tensor):
            tensor.wait_ge(gw_sem, 1)
            for _ in range(NWARM):
                nc.tensor.matmul(
                    pts[0].ap()[:KCAP, :WNT],
                    gw.ap()[:, :KCAP],
                    gw.ap()[:, :WNT],
                    start=True, stop=True,
                )
            tensor.wait_ge(pre_sem, 16)
            bmm = None
            for p in range(npairs):
                ns = nslots_of(p)
                for h in range(ns):
                    s = 2 * p + h
                    bmm = nc.tensor.matmul(
                        pts[3].ap()[h * 64:h * 64 + 64, p:p + 1],
                        wtap(s),
                        ones.ap()[:, 0:1],
                        start=True, stop=True,
                        tile_position=(0, 64 * h) if ns == 2 else None,
                    )
            bmm.then_inc(bmm_sem, 1)

            def need(s):
                if s == 0:
                    tensor.wait_ge(pre_sem, 16)
                elif s % 2 == 0:
                    tensor.wait_ge(ine_sem, 16 * (s // 2))
                else:
                    tensor.wait_ge(ino_sem, 16 * ((s - 1) // 2 + 1))

            for p in range(npairs):
                ns = nslots_of(p)
                for h in range(ns):
                    s = 2 * p + h
                    need(s)
                    for c in range(4):
                        t = 4 * p + c
                        if h == 0 and t >= 4:
                            tprev = t - 4
                            if tprev % 2 == 0:
                                tensor.wait_ge(cpv, tprev // 2 + 1)
                            else:
                                tensor.wait_ge(cps, tprev // 2 + 1)
                        if h == 0 and t == 3:
                            tensor.wait_ge(bias_sem, 1)  # pts[3] freed
                        last = None
                        for n in range(PS // NT):
                            col = W0 + s * HW + c * PS + n * NT
                            last = nc.tensor.matmul(
                                pts[c].ap()[h * 64:h * 64 + 64,
                                            n * NT:(n + 1) * NT],
                                wtap(s),
                                rt.ap()[:F, col:col + NT],
                                start=True, stop=True,
                                tile_position=(0, 64 * h) if ns == 2
                                else None,
                            )
                        if h == ns - 1:
                            last.then_inc(mm_sem, 1)

        @block.vector
        def _(vector):
            vector.memset(ones.ap()[:, :], 0.5)
            vector.memset(gw.ap()[:, :], 0.25).then_inc(gw_sem, 1)
            vector.wait_ge(bmm_sem, 1)
            vector.tensor_scalar(bs.ap()[:, :], pts[3].ap()[:, :npairs],
                                 -OOFF, None, mybir.AluOpType.add,
                                 ).then_inc(bias_sem, 1)
            nmain = ntiles - 4 if fin_half else ntiles
            for t in range(0, nmain, 2):
                p, c = divmod(t, 4)
                ns = nslots_of(p)
                vector.wait_ge(mm_sem, t + 1)
                og = ogs[p % NOG]
                vector.tensor_scalar(og.ap()[:64 * ns, c * PS:c * PS + PS],
                                     pts[c].ap()[:64 * ns, :],
                                     bs.ap()[:64 * ns, p:p + 1],
                                     None,
                                     mybir.AluOpType.add,
                                     ).then_inc(cpv, 1)
            if fin_half:
                pf = npairs - 1
                og = ogs[pf % NOG]
                for c in range(4):
                    vector.wait_ge(mm_sem, 4 * pf + c + 1)
                    vector.tensor_scalar(og.ap()[:64, c * PS:c * PS + NT],
                                         pts[c].ap()[:64, :NT],
                                         bs.ap()[:64, pf:pf + 1],
                                         None,
                                         mybir.AluOpType.add,
                                         ).then_inc(cpv, 1)

        @block.gpsimd
        def _(gpsimd):
            for p in range(npairs):
                if out_q(p) == 0:
                    issue_out(gpsimd, p)
            gpsimd.wait_ge(out_done, 16 * n_out)

    nc.all_engine_barrier()
    nc.clear_and_free_semaphores([gw_sem, bmm_sem, bias_sem, pre_sem,
                                  ine_sem, ino_sem, mm_sem, cpv, cps,
                                  out_done])

    nc._raw_ctx = ctx
    _NC_CACHE[key] = nc
    return nc


def kernel(font_pred, char_labels, char_rec_vec, text_indexes, alpha_table):
    global LAST_RESULT
    BT = font_pred.shape[0] * font_pred.shape[1]

    fp = np.asarray(font_pred, np.float32).reshape(BT, F)
    m = fp.max(axis=1, keepdims=True)
    e = np.exp(fp - m)
    sfm = e / e.sum(axis=1, keepdims=True)
    topk = np.argpartition(-fp, TOPK - 1, axis=1)[:, :TOPK]
    M = np.zeros((BT, F), np.float32)
    rows = np.arange(BT)[:, None]
    M[rows, topk] = sfm[rows, topk]

    char_idx = np.asarray(char_rec_vec).argmax(axis=1)
    ti = np.asarray(text_indexes).reshape(-1)
    Wc = M[ti] * np.float32(OS)

    chunks = []
    order = np.argsort(char_idx, kind="stable")
    sorted_cls = char_idx[order]
    starts = np.searchsorted(sorted_cls, np.arange(C), side="left")
    ends = np.searchsorted(sorted_cls, np.arange(C), side="right")
    for c in range(C):
        ids = order[starts[c]:ends[c]]
        for i in range(0, len(ids), KCAP):
            chunks.append((c, ids[i:i + KCAP]))
    while len(chunks) % NCORES:
        k = max(range(len(chunks)), key=lambda i: len(chunks[i][1]))
        c, ids = chunks[k]
        if len(ids) < 2:
            chunks.append((c, np.array([], np.int64)))
            continue
        h = len(ids) // 2
        chunks[k] = (c, ids[:h])
        chunks.append((c, ids[h:]))
    S = len(chunks) // NCORES

    chunks.sort(key=lambda ch: -len(ch[1]))
    per_core = [[chunks[NCORES * j + i] for j in range(S)]
                for i in range(NCORES)]
    W0 = S * WTB

    tbl = np.asarray(alpha_table, np.float32).reshape(F, C, HW)
    tbl8 = (tbl * np.float32(1.0 / 255.0) - np.float32(0.5)).astype(E3M4)

    in_maps = []
    slot_ids = []
    for core in range(NCORES):
        table_i = np.zeros((P128, W0 + S * HW), E3M4)
        lhsT_i = np.zeros((F, S * KCAP), np.float32)
        ids_i = []
        for s, (c, ids) in enumerate(per_core[core]):
            table_i[:F, W0 + s * HW:W0 + (s + 1) * HW] = tbl8[:, c, :]
            if len(ids):
                lhsT_i[:, s * KCAP:s * KCAP + len(ids)] = Wc[ids].T
            ids_i.append(ids)
        table_i[:F, :W0] = lhsT_i.astype(BF16).view(E3M4)
        in_maps.append({"table": table_i})
        slot_ids.append(ids_i)

    nc = _build(S)
    res = run_bass_kernel_spmd(nc, in_maps, core_ids=list(range(NCORES)))
    LAST_RESULT = res

    inv = np.float32(1.0 / OS)
    off = np.float32(OOFF / OS)
    out_full = np.zeros((N, HW), np.float32)
    for core in range(NCORES):
        o = np.asarray(res.results[core]["out"]).astype(np.float32)
        for s, ids in enumerate(slot_ids[core]):
            if len(ids):
                out_full[ids] = o[64 * s:64 * s + len(ids), :] * inv + off
    return out_full.reshape(N, 1, 1, 64, 64)


# revision 10
# speedup vs baseline: 1.0843x; 1.0843x over previous
"""AlphaRenderer v9 (C3): P128 input, int8 out, dual-ring per-slot feed.

Math: out = W @ e3m4(A/255-0.5) + 0.5*sum(W); weights pre-scaled by
OS=400 on host; og int8 = OS*out - OOFF (ACT/DVE round+saturate);
host un-scales. Table padded to 128 partitions so HWDGE DMAs spread
across all 16 SDMA engines (measured 399 vs 142 GB/s).

Schedule:
- sync ring: weight-prefix DMA first (bias matmuls start ~4us
  earlier), then even slots; scalar ring: odd slots. Per-ring FIFO
  completion -> two counting sems.
- Warmup NWARM matmuls bridge the pre-input idle window so the HAM
  clock gate (4096cy window) reaches 8/8 before real matmuls.
- Outputs: gpsimd ring even pairs, scalar odd, sync final; final
  pair in 4 column segments to cut the post-matmul tail. NOG=npairs
  og buffers -> no reuse gating; single out_done semaphore.
"""
from contextlib import ExitStack

import ml_dtypes
import numpy as np

import concourse.bass as bass
import concourse.mybir as mybir
from concourse.bass_utils import run_bass_kernel_spmd

BF16 = np.dtype(ml_dtypes.bfloat16)
E3M4 = np.dtype(ml_dtypes.float8_e3m4)

NCORES = 8
F = 100
C = 100
N = 4096
HW = 4096
TOPK = 20
KCAP = 64
NT = 512
PS = 1024
WTB = 2 * KCAP
P128 = 128

OS = 400.0       # output int8 scale
OOFF = 123.2     # og = OS*out - OOFF

_NC_CACHE: dict = {}
LAST_RESULT = None
NWARM = 18
WNT = 256


def _build(S):
    key = ("v12", S)
    if key in _NC_CACHE:
        return _NC_CACHE[key]
    dt8 = mybir.dt.float8e3
    dtb = mybir.dt.bfloat16
    npairs = (S + 1) // 2
    NOG = npairs
    ntiles = 4 * npairs
    nslots_of = lambda p: min(2, S - 2 * p)
    R = 64 * S
    W0 = S * WTB

    nc = bass.Bass("TRN2", target_bir_lowering=False, debug=False,
                   num_devices=NCORES)
    table = nc.dram_tensor("table", [P128, W0 + S * HW], dt8,
                           kind="ExternalInput").ap()
    out = nc.dram_tensor("out", [R, HW], mybir.dt.int8,
                         kind="ExternalOutput").ap()

    ctx = ExitStack()
    ones = ctx.enter_context(nc.sbuf_tensor("ones", [F, 1], dtb))
    gw = ctx.enter_context(nc.sbuf_tensor("gw", [F, NT], dtb))
    bs = ctx.enter_context(nc.sbuf_tensor("bs", [128, npairs],
                                          mybir.dt.float32))
    rt = ctx.enter_context(nc.sbuf_tensor("rt", [P128, W0 + S * HW], dt8))
    ogs = [ctx.enter_context(nc.sbuf_tensor(f"og{i}", [128, HW],
                                            mybir.dt.int8))
           for i in range(NOG)]
    pts = [ctx.enter_context(nc.psum_tensor(f"pt{i}", [128, PS],
                                            mybir.dt.float32))
           for i in range(4)]
    gw_sem = ctx.enter_context(nc.semaphore("gw_sem"))
    bmm_sem = ctx.enter_context(nc.semaphore("bmm_sem"))
    bias_sem = ctx.enter_context(nc.semaphore("bias_sem"))
    pre_sem = ctx.enter_context(nc.semaphore("pre_sem"))
    ine_sem = ctx.enter_context(nc.semaphore("ine_sem"))
    ino_sem = ctx.enter_context(nc.semaphore("ino_sem"))
    mm_sem = ctx.enter_context(nc.semaphore("mm_sem"))
    cpv = ctx.enter_context(nc.semaphore("cpv"))
    cps = ctx.enter_context(nc.semaphore("cps"))
    out_done = ctx.enter_context(nc.semaphore("out_done"))

    def wtap(s):
        return rt.ap()[:F, s * WTB:(s + 1) * WTB].bitcast(dtb)

    fin_half = nslots_of(npairs - 1) == 1

    def out_segs(p):
        if nslots_of(p) == 2:
            return [(0, HW, 4)]
        return [(j * PS, PS, j + 1) for j in range(4)]

    n_out = sum(len(out_segs(p)) for p in range(npairs))

    def issue_out(eng, p):
        ns = nslots_of(p)
        for (off, wid, hi4) in out_segs(p):
            if p == npairs - 1 and fin_half:
                eng.wait_ge(cpv, 2 * (npairs - 1) + hi4)
                eng.wait_ge(cps, 2 * (npairs - 1) + hi4)
            else:
                hi = 4 * p + hi4 - 1         # last tile index needed
                eng.wait_ge(cpv, hi // 2 + 1)
                eng.wait_ge(cps, (hi + 1) // 2)
            eng.dma_start(
                out[128 * p:128 * p + 64 * ns, off:off + wid],
                ogs[p % NOG].ap()[:64 * ns, off:off + wid]
            ).then_inc(out_done, 16)

    # pair -> out ring: 0=gpsimd 1=scalar 2=sync(final)
    def out_q(p):
        if p == npairs - 1:
            return 2
        return 0 if p % 2 == 0 else 1

    with nc.Block() as block:

        @block.sync
        def _(sync):
            sync.dma_start(rt[:, 0:W0 + HW],
                           table[:, 0:W0 + HW]).then_inc(pre_sem, 16)
            for s in range(2, S, 2):
                lo = W0 + s * HW
                sync.dma_start(rt[:, lo:lo + HW],
                               table[:, lo:lo + HW]).then_inc(ine_sem, 16)
            for p in range(npairs):
                if out_q(p) == 2:
                    issue_out(sync, p)

        @block.scalar
        def _(scalar):
            for s in range(1, S, 2):
                lo = W0 + s * HW
                scalar.dma_start(rt[:, lo:lo + HW],
                                 table[:, lo:lo + HW]).then_inc(ino_sem, 16)
            nmain = ntiles - 4 if fin_half else ntiles
            for t in range(1, nmain, 2):
                p, c = divmod(t, 4)
                ns = nslots_of(p)
                scalar.wait_ge(mm_sem, t + 1)
                if t == 1:
                    scalar.wait_ge(bias_sem, 1)
                og = ogs[p % NOG]
                scalar.activation(og.ap()[:64 * ns, c * PS:c * PS + PS],
                                  pts[c].ap()[:64 * ns, :],
                                  mybir.ActivationFunctionType.Identity,
                                  bias=bs.ap()[:64 * ns, p:p + 1],
                                  scale=1.0).then_inc(cps, 1)
                if c == 3 and out_q(p) == 1:
                    issue_out(scalar, p)
            if fin_half:
                pf = npairs - 1
                og = ogs[pf % NOG]
                for c in range(4):
                    scalar.wait_ge(mm_sem, 4 * pf + c + 1)
                    scalar.activation(
                        og.ap()[:64, c * PS + NT:c * PS + PS],
                        pts[c].ap()[:64, NT:PS],
                        mybir.ActivationFunctionType.Identity,
                        bias=bs.ap()[:64, pf:pf + 1],
                        scale=1.0).then_inc(cps, 1)

        @block.tensor
        def _(tensor):
            tensor.wait_ge(gw_sem, 1)
            for _ in range(NWARM):
                nc.tensor.matmul(
                    pts[0].ap()[:KCAP, :WNT],
                    gw.ap()[:, :KCAP],
                    gw.ap()[:, :WNT],
                    start=True, stop=True,
                )
            tensor.wait_ge(pre_sem, 16)
            bmm = None
            for p in range(npairs):
                ns = nslots_of(p)
                for h in range(ns):
                    s = 2 * p + h
                    bmm = nc.tensor.matmul(
                        pts[3].ap()[h * 64:h * 64 + 64, p:p + 1],
                        wtap(s),
                        ones.ap()[:, 0:1],
                        start=True, stop=True,
                        tile_position=(0, 64 * h) if ns == 2 else None,
                    )
            bmm.then_inc(bmm_sem, 1)

            def need(s):
                if s == 0:
                    tensor.wait_ge(pre_sem, 16)
                elif s % 2 == 0:
                    tensor.wait_ge(ine_sem, 16 * (s // 2))
                else:
                    tensor.wait_ge(ino_sem, 16 * ((s - 1) // 2 + 1))

            for p in range(npairs):
                ns = nslots_of(p)
                for h in range(ns):
                    s = 2 * p + h
                    need(s)
                    for c in range(4):
                        t = 4 * p + c
                        if h == 0 and t >= 4:
                            tprev = t - 4
                            if tprev % 2 == 0:
                                tensor.wait_ge(cpv, tprev // 2 + 1)
                            else:
                                tensor.wait_ge(cps, tprev // 2 + 1)
                        if h == 0 and t == 3:
                            tensor.wait_ge(bias_sem, 1)  # pts[3] freed
                        last = None
                        for n in range(PS // NT):
                            col = W0 + s * HW + c * PS + n * NT
                            last = nc.tensor.matmul(
                                pts[c].ap()[h * 64:h * 64 + 64,
                                            n * NT:(n + 1) * NT],
                                wtap(s),
                                rt.ap()[:F, col:col + NT],
                                start=True, stop=True,
                                tile_position=(0, 64 * h) if ns == 2
                                else None,
                            )
                        if h == ns - 1:
                            last.then_inc(mm_sem, 1)

        @block.vector
        def _(vector):
            vector.memset(ones.ap()[:, :], 0.5)
            vector.memset(gw.ap()[:, :], 0.25).then_inc(gw_sem, 1)
            vector.wait_ge(bmm_sem, 1)
            vector.tensor_scalar(bs.ap()[:, :], pts[3].ap()[:, :npairs],
                                 -OOFF, None, mybir.AluOpType.add,
                                 ).then_inc(bias_sem, 1)
            nmain = ntiles - 4 if fin_half else ntiles
            for t in range(0, nmain, 2):
                p, c = divmod(t, 4)
                ns = nslots_of(p)
                vector.wait_ge(mm_sem, t + 1)
                og = ogs[p % NOG]
                vector.tensor_scalar(og.ap()[:64 * ns, c * PS:c * PS + PS],
                                     pts[c].ap()[:64 * ns, :],
                                     bs.ap()[:64 * ns, p:p + 1],
                                     None,
                                     mybir.AluOpType.add,
                                     ).then_inc(cpv, 1)
            if fin_half:
                pf = npairs - 1
                og = ogs[pf % NOG]
                for c in range(4):
                    vector.wait_ge(mm_sem, 4 * pf + c + 1)
                    vector.tensor_scalar(og.ap()[:64, c * PS:c * PS + NT],
                                         pts[c].ap()[:64, :NT],
                                         bs.ap()[:64, pf:pf + 1],
                                         None,
                                         mybir.AluOpType.add,
                                         ).then_inc(cpv, 1)

        @block.gpsimd
        def _(gpsimd):
            for p in range(npairs):
                if out_q(p) == 0:
                    issue_out(gpsimd, p)
            gpsimd.wait_ge(out_done, 16 * n_out)

    nc.all_engine_barrier()
    nc.clear_and_free_semaphores([gw_sem, bmm_sem, bias_sem, pre_sem,
                                  ine_sem, ino_sem, mm_sem, cpv, cps,
                                  out_done])

    nc._raw_ctx = ctx
    _NC_CACHE[key] = nc
    return nc


def kernel(font_pred, char_labels, char_rec_vec, text_indexes, alpha_table):
    global LAST_RESULT
    BT = font_pred.shape[0] * font_pred.shape[1]

    fp = np.asarray(font_pred, np.float32).reshape(BT, F)
    m = fp.max(axis=1, keepdims=True)
    e = np.exp(fp - m)
    sfm = e / e.sum(axis=1, keepdims=True)
    topk = np.argpartition(-fp, TOPK - 1, axis=1)[:, :TOPK]
    M = np.zeros((BT, F), np.float32)
    rows = np.arange(BT)[:, None]
    M[rows, topk] = sfm[rows, topk]

    char_idx = np.asarray(char_rec_vec).argmax(axis=1)
    ti = np.asarray(text_indexes).reshape(-1)
    Wc = M[ti] * np.float32(OS)

    chunks = []
    order = np.argsort(char_idx, kind="stable")
    sorted_cls = char_idx[order]
    starts = np.searchsorted(sorted_cls, np.arange(C), side="left")
    ends = np.searchsorted(sorted_cls, np.arange(C), side="right")
    for c in range(C):
        ids = order[starts[c]:ends[c]]
        for i in range(0, len(ids), KCAP):
            chunks.append((c, ids[i:i + KCAP]))
    while len(chunks) % NCORES:
        k = max(range(len(chunks)), key=lambda i: len(chunks[i][1]))
        c, ids = chunks[k]
        if len(ids) < 2:
            chunks.append((c, np.array([], np.int64)))
            continue
        h = len(ids) // 2
        chunks[k] = (c, ids[:h])
        chunks.append((c, ids[h:]))
    S = len(chunks) // NCORES

    chunks.sort(key=lambda ch: -len(ch[1]))
    per_core = [[chunks[NCORES * j + i] for j in range(S)]
                for i in range(NCORES)]
    W0 = S * WTB

    tbl = np.asarray(alpha_table, np.float32).reshape(F, C, HW)
    tbl8 = (tbl * np.float32(1.0 / 255.0) - np.float32(0.5)).astype(E3M4)

    in_maps = []
    slot_ids = []
    for core in range(NCORES):
        table_i = np.zeros((P128, W0 + S * HW), E3M4)
        lhsT_i = np.zeros((F, S * KCAP), np.float32)
        ids_i = []
        for s, (c, ids) in enumerate(per_core[core]):
            table_i[:F, W0 + s * HW:W0 + (s + 1) * HW] = tbl8[:, c, :]
            if len(ids):
                lhsT_i[:, s * KCAP:s * KCAP + len(ids)] = Wc[ids].T
            ids_i.append(ids)
        table_i[:F, :W0] = lhsT_i.astype(BF16).view(E3M4)
        in_maps.append({"table": table_i})
        slot_ids.append(ids_i)

    nc = _build(S)
    res = run_bass_kernel_spmd(nc, in_maps, core_ids=list(range(NCORES)))
    LAST_RESULT = res

    inv = np.float32(1.0 / OS)
    off = np.float32(OOFF / OS)
    out_full = np.zeros((N, HW), np.float32)
    for core in range(NCORES):
        o = np.asarray(res.results[core]["out"]).astype(np.float32)
        for s, ids in enumerate(slot_ids[core]):
            if len(ids):
                out_full[ids] = o[64 * s:64 * s + len(ids), :] * inv + off
    return out_full.reshape(N, 1, 1, 64, 64)
